# revision 9
# baseline (speedup 1.0000x reference)
"""MHA (projections + masked softmax attention) on 8 NeuronCores.

Data-parallel over batch (B=8 -> 1 batch element per core, no collectives).
bf16 matmul operands (fp32 PSUM accumulation + fp32 softmax normalization).

Per core, transposed layout:
  V  = x_v  @ Wv      [Sk, D]   (first: its output is needed by every head)
  KT = Wk^T @ x_k^T   [D, Sk]
  QT = Wq^T @ x_q^T   [D, Sq]

Attention per head-pair (2p, 2p+1) in "scores transposed" layout S^T[k, q]:
  the two heads' score matmuls use PE row-groups 0-63 / 64-127 and run
  concurrently (row tiling). exp(0.125*s) runs unmasked (scores ~N(0,1),
  no overflow), then e *= mask01 (bf16) zeroes the masked transition range.
  O^T[d,q] & Z[q] in ONE accumulating matmul per 512-query half:
  lhsT = [V_h | ones] (65 cols); raw [O^T ; Z] is DMA'd out and the host
  divides / transposes / un-sorts.

The PE is a strict FIFO, so projection matmuls for chunk p+1 are
zip-interleaved into the attention emission of pair p: the PE fills
exp-latency waits with independent projection work and the HAM clock
stays warm.

PSUM budget (8 banks): proj acc 2x[128,512] + pair scores 2x[128,1024] +
attention out 2x[65,512].

Host: transposes, sort queries by valid_len (column-suffix skipping of
fully-masked key chunks at 32-col granularity + narrow mask ranges),
bf16 0/1 mask, exact fixup of valid_len==0 rows.
"""

import os
import sys

if "/opt/trn_rl_repo" not in sys.path:
    sys.path.insert(0, "/opt/trn_rl_repo")

import numpy as np

ABLATE = set(os.environ.get("ABLATE", "").split(","))

B, S, D, H = 8, 1024, 1024, 16
DH = D // H  # 64
P = 128
HB = 512  # half-width of the query range (1 fp32 PSUM bank)
KC = S // P  # 8 key chunks
DC = D // P  # 8 hidden chunks
N_CORES = 8


def _build_nc(col_start, pred_end, reps=1):
    """col_start[kc]: first sorted-q column (mult of 32, 0..1024) needing
    key-chunk kc (1024 = chunk skipped). pred_end[kc]: end (exclusive, mult
    of 32) of the masked transition range. Unions over cores. col_start[0]
    must be 0."""
    from contextlib import ExitStack

    import concourse.mybir as mybir
    import concourse.tile as tile
    from concourse import bacc

    fp32 = mybir.dt.float32
    bf16 = mybir.dt.bfloat16
    AF = mybir.ActivationFunctionType

    nc = bacc.Bacc(
        "TRN2",
        target_bir_lowering=False,
        debug=False,
        enable_asserts=False,
        num_devices=N_CORES,
    )

    xqT = nc.dram_tensor("xqT", (D, S), bf16, kind="ExternalInput").ap()
    xkT = nc.dram_tensor("xkT", (D, S), bf16, kind="ExternalInput").ap()
    xvT = nc.dram_tensor("xvT", (D, S), bf16, kind="ExternalInput").ap()
    wq = nc.dram_tensor("wq", (D, D), bf16, kind="ExternalInput").ap()
    wk = nc.dram_tensor("wk", (D, D), bf16, kind="ExternalInput").ap()
    wv = nc.dram_tensor("wv", (D, D), bf16, kind="ExternalInput").ap()
    maskb = nc.dram_tensor("maskb", (S, S), bf16, kind="ExternalInput").ap()
    # raw per-head output: rows 0:64 = unnormalized O^T, row 64 = Z
    outT = nc.dram_tensor("outT", (H, DH + 1, S), fp32, kind="ExternalOutput").ap()

    with ExitStack() as ctx:
        tc = ctx.enter_context(tile.TileContext(nc))
        const = ctx.enter_context(tc.tile_pool(name="const", bufs=1))
        persist = ctx.enter_context(tc.tile_pool(name="persist", bufs=1))
        wpool = ctx.enter_context(tc.tile_pool(name="wpool", bufs=1))
        ppool = ctx.enter_context(tc.tile_pool(name="ppool", bufs=1, space="PSUM"))
        epool = ctx.enter_context(tc.tile_pool(name="epool", bufs=18))
        mpool = ctx.enter_context(tc.tile_pool(name="mpool", bufs=3))

        def mm(out_ap, lhsT, rhs, base, start, stop):
            # split a wide matmul into <=512-col pieces so each PE write
            # stays inside one PSUM bank. base = column offset of out_ap[0]
            # within its tile (bank alignment reference).
            w = rhs.shape[-1]
            off = 0
            while off < w:
                step = min(HB - ((base + off) % HB), w - off)
                nc.tensor.matmul(
                    out_ap[:, off : off + step],
                    lhsT,
                    rhs[:, off : off + step],
                    start=start,
                    stop=stop,
                )
                off += step

        wrm = const.tile([P, HB], bf16)
        nc.vector.memset(wrm[:], 0.5)

        rep_cm = (
            tc.For_i(0, reps, 1, hint_engines=(mybir.EngineType.PE,))
            if reps > 1
            else None
        )
        if rep_cm is not None:
            ctx.enter_context(rep_cm)

        # HAM warmup: back-to-back dummy matmuls keep the PE busy through
        # the initial input-DMA window so the clock gate opens (K=8/8)
        # before the real matmul stream starts.
        if "nowarm" not in ABLATE:
            wps = ppool.tile([P, HB], fp32, tag="sc0", name="warm")
            for _ in range(20):
                nc.tensor.matmul(wps[:], wrm[:, :P], wrm[:], start=True, stop=True)

        qt_sb = [persist.tile([P, S], bf16, tag=f"qt{i}", name=f"qt{i}") for i in range(DC)]
        kt_sb = [persist.tile([P, S], bf16, tag=f"kt{i}", name=f"kt{i}") for i in range(DC)]
        va_sb = [persist.tile([P, H * (DH + 1)], bf16, tag=f"va{i}", name=f"va{i}") for i in range(KC)]
        mk_sb = persist.tile([P, KC * S], bf16, tag="mk", name="mk")
        mk3 = mk_sb.rearrange("p (kc q) -> p kc q", q=S)

        # ---- inputs: coarse DMAs (chunk dc at cols dc*S); xv/wv split in
        # halves so the first V-projection matmuls start ~5us earlier ----
        def load_big(dram, nm, pieces=1):
            t = wpool.tile([P, DC * S], bf16, tag=nm, name=nm)
            t3 = t.rearrange("p (dc q) -> p dc q", q=S)
            d3 = dram.rearrange("(dc p) q -> p dc q", p=P)
            h = DC // pieces
            aps = []
            for i in range(pieces):
                aps.append((t3[:, i * h : (i + 1) * h, :], d3[:, i * h : (i + 1) * h, :]))
            return t, aps

        xv2, xv_aps = load_big(xvT, "xv", 2)
        wv2, wv_aps = load_big(wv, "wv", 2)
        for i in range(2):
            nc.sync.dma_start(*xv_aps[i])
            nc.sync.dma_start(*wv_aps[i])
        xk2, xk_aps = load_big(xkT, "xk")
        nc.sync.dma_start(*xk_aps[0])
        wk2, wk_aps = load_big(wk, "wk")
        nc.sync.dma_start(*wk_aps[0])
        mkb3 = maskb.rearrange("(kc p) q -> p kc q", p=P)
        nc.sync.dma_start(mk3[:, 0 : KC // 2, :], mkb3[:, 0 : KC // 2, :])
        xq2, xq_aps = load_big(xqT, "xq")
        nc.sync.dma_start(*xq_aps[0])
        wq2, wq_aps = load_big(wq, "wq")
        nc.sync.dma_start(*wq_aps[0])
        nc.sync.dma_start(mk3[:, KC // 2 : KC, :], mkb3[:, KC // 2 : KC, :])

        kcs = [kc for kc in range(KC) if col_start[kc] < S]
        nbuf = [0]
        abuf = [0]

        # ---- V projection: out[k, d] per key chunk, 512-col halves ----
        def vproj(kc):
            va3 = va_sb[kc].rearrange("p (h d) -> p h d", d=DH + 1)
            nc.vector.memset(va3[:, :, DH], 1.0)
            for half in range(2):
                acc = ppool.tile([P, HB], fp32, tag=f"pj{nbuf[0] % 2}", name="vacc")
                nbuf[0] += 1
                for dc in range(DC):
                    nc.tensor.matmul(
                        acc[:],
                        xv2[:, dc * S + kc * P : dc * S + (kc + 1) * P],
                        wv2[:, dc * D + half * HB : dc * D + (half + 1) * HB],
                        start=(dc == 0),
                        stop=(dc == DC - 1),
                    )
                nc.scalar.copy(
                    va3[:, half * (H // 2) : (half + 1) * (H // 2), 0:DH],
                    acc[:].rearrange("p (h d) -> p h d", d=DH),
                )

        def proj_ops(p):
            """Closures emitting the K+Q projections of chunk p in ~2-MM
            steps, for zip-interleaving into the attention emission."""
            ops = []
            for w2, x2, dst in ((wk2, xk2, kt_sb), (wq2, xq2, qt_sb)):
                for half in range(2):
                    box = {}
                    for dc2 in range(0, DC, 2):
                        def step(w2=w2, x2=x2, dst=dst, half=half, dc2=dc2, box=box):
                            if dc2 == 0:
                                box["acc"] = ppool.tile(
                                    [P, HB], fp32, tag=f"pj{nbuf[0] % 2}", name="acc"
                                )
                                nbuf[0] += 1
                            for dc in (dc2, dc2 + 1):
                                nc.tensor.matmul(
                                    box["acc"][:],
                                    w2[:, dc * D + p * P : dc * D + (p + 1) * P],
                                    x2[:, dc * S + half * HB : dc * S + (half + 1) * HB],
                                    start=(dc == 0),
                                    stop=(dc == DC - 1),
                                )
                            if dc2 == DC - 2:
                                nc.vector.tensor_copy(
                                    dst[p][:, half * HB : (half + 1) * HB],
                                    box["acc"][:],
                                )
                        ops.append(step)
            return ops

        def attend_pair(p, fill):
            """Heads (2p, 2p+1): row-packed scores, exp, mask-mul, AV.
            `fill` = list of closures (projection work) drained into the
            emission to keep the PE FIFO busy during exp waits."""
            oc = p
            es = {0: {}, 1: {}}
            fi = 0

            def emit_mm(kc, ro):
                # head 2p on PE rows 0:64, head 2p+1 on rows 64:128 — an
                # adjacent (ro=0, ro=1) pair co-dispatches onto disjoint
                # row-groups and runs concurrently.
                c0 = col_start[kc]
                sc = ppool.tile([P, S], fp32, tag=f"sc{ro}", name="sc")
                mm(sc[:, c0:], kt_sb[oc][ro * DH : (ro + 1) * DH, kc * P : (kc + 1) * P],
                   qt_sb[oc][ro * DH : (ro + 1) * DH, c0:], c0, True, True)
                return sc

            def emit_post(kc, ro, sc):
                c0 = col_start[kc]
                cv = pred_end[kc]
                e = epool.tile([P, S], bf16, tag="e")
                nc.scalar.activation(e[:, c0:], sc[:, c0:], AF.Exp, scale=0.125)
                if cv > c0 and "nopred" not in ABLATE:
                    nc.vector.tensor_mul(e[:, c0:cv], e[:, c0:cv], mk3[:, kc, c0:cv])
                es[ro][kc] = e

            # The ro=1 score stream lags one chunk and its exp is emitted
            # FIRST on ACT, so by the time exp_e(kc-1) frees sc0, sc1 has
            # long been free: both next matmuls are ready together and the
            # PE overlaps them (row tiling).
            prev = None
            sco_prev = None
            for kc in kcs:
                sce = emit_mm(kc, 0)
                if prev is not None:
                    sco = emit_mm(prev, 1)
                    emit_post(prev, 1, sco)
                emit_post(kc, 0, sce)
                prev = kc
                for _ in range(2):
                    if fi < len(fill):
                        fill[fi]()
                        fi += 1
            sco = emit_mm(prev, 1)
            emit_post(prev, 1, sco)
            while fi < len(fill):
                fill[fi]()
                fi += 1
            for ro in (0, 1):
                h = 2 * p + ro
                for half in range(2):
                    lo, hi = half * HB, (half + 1) * HB
                    ks = [kc for kc in kcs if col_start[kc] < hi]
                    at = ppool.tile(
                        [DH + 1, HB], fp32, tag=f"at{abuf[0] % 2}", name="at"
                    )
                    abuf[0] += 1
                    for j, kc in enumerate(ks):
                        c0 = max(col_start[kc], lo)
                        nc.tensor.matmul(
                            at[:, c0 - lo :],
                            va_sb[kc][:, h * (DH + 1) : (h + 1) * (DH + 1)],
                            es[ro][kc][:, c0:hi],
                            start=(j == 0),
                            stop=(j == len(ks) - 1),
                        )
                    asb = mpool.tile([DH + 1, HB], fp32, tag="asb")
                    nc.vector.tensor_copy(asb[:], at[:])
                    nc.sync.dma_start(outT[h][:, lo:hi], asb[:])

        if "noproj" not in ABLATE:
            for kc in range(KC):
                vproj(kc)
            for op in proj_ops(0):
                op()
            if "noattn" not in ABLATE:
                for p in range(1, DC):
                    attend_pair(p - 1, proj_ops(p))
                attend_pair(DC - 1, [])

    nc.compile()
    return nc


_NC_CACHE = {}
_LAST_IN_MAPS = None


def _get_nc(col_start, pred_end):
    key = (tuple(col_start), tuple(pred_end))
    if key not in _NC_CACHE:
        _NC_CACHE[key] = _build_nc(list(col_start), list(pred_end))
    return _NC_CACHE[key]


def _prep(query, key, value, valid_len, Wq, Wk, Wv):
    import ml_dtypes

    bf = ml_dtypes.bfloat16
    kidx = np.arange(S, dtype=np.int32)
    orders = []
    in_maps = []
    col_start = [S] * KC
    pred_end = [0] * KC
    wqb, wkb, wvb = Wq.astype(bf), Wk.astype(bf), Wv.astype(bf)
    for b in range(B):
        vl = valid_len[b]
        vl2 = np.where(vl == 0, 1, vl).astype(np.int32)
        order = np.argsort(vl2, kind="stable")
        orders.append(order)
        vs = vl2[order]
        for kc in range(KC):
            need = vs > (kc * P)
            c0 = S if not need.any() else (int(np.argmax(need)) // 32) * 32
            col_start[kc] = min(col_start[kc], c0)
            full = vs >= ((kc + 1) * P)
            cv = S if not full.any() else int(np.argmax(full))
            pred_end[kc] = max(pred_end[kc], min(S, -(-cv // 32) * 32))
        in_maps.append(
            {
                "xqT": np.ascontiguousarray(query[b][order].T.astype(bf)),
                "xkT": np.ascontiguousarray(key[b].T.astype(bf)),
                "xvT": np.ascontiguousarray(value[b].T.astype(bf)),
                "wq": wqb,
                "wk": wkb,
                "wv": wvb,
                "maskb": (kidx[:, None] < vs[None, :]).astype(bf),
            }
        )
    return in_maps, orders, col_start, pred_end


def kernel(query, key, value, valid_len, Wq, Wk, Wv):
    from concourse import bass_utils

    query = np.asarray(query, dtype=np.float32)
    key = np.asarray(key, dtype=np.float32)
    value = np.asarray(value, dtype=np.float32)
    valid_len = np.asarray(valid_len, dtype=np.int32)
    Wq = np.asarray(Wq, dtype=np.float32)
    Wk = np.asarray(Wk, dtype=np.float32)
    Wv = np.asarray(Wv, dtype=np.float32)

    in_maps, orders, col_start, pred_end = _prep(
        query, key, value, valid_len, Wq, Wk, Wv
    )
    nc = _get_nc(col_start, pred_end)
    global _LAST_IN_MAPS
    _LAST_IN_MAPS = in_maps
    res = bass_utils.run_bass_kernel_spmd(nc, in_maps, core_ids=list(range(N_CORES)))

    outs = np.empty((B, S, D), dtype=np.float32)
    for b in range(B):
        raw = res.results[b]["outT"]  # [H, DH+1, S] sorted-query order
        o = raw[:, :DH, :]  # [H, DH, S]
        z = raw[:, DH, :]  # [H, S]
        o_sorted = (o / z[:, None, :]).transpose(2, 0, 1).reshape(S, D)
        inv = np.empty(S, dtype=np.int64)
        inv[orders[b]] = np.arange(S)
        outs[b] = o_sorted[inv]
        zrows = np.where(valid_len[b] == 0)[0]
        if len(zrows):
            outs[b][zrows] = value[b].mean(axis=0) @ Wv
    return outs


# revision 10
# speedup vs baseline: 1.0712x; 1.0712x over previous
"""MHA (projections + masked softmax attention) on 8 NeuronCores.

Data-parallel over batch (B=8 -> 1 batch element per core, no collectives).
bf16 matmul operands (fp32 PSUM accumulation + fp32 softmax normalization).

Per core, transposed layout:
  V  = x_v  @ Wv      [Sk, D]   (first: its output is needed by every head)
  KT = Wk^T @ x_k^T   [D, Sk]
  QT = Wq^T @ x_q^T   [D, Sq]

Attention per head-pair (2p, 2p+1) in "scores transposed" layout S^T[k, q]:
  the two heads' score matmuls use PE row-groups 0-63 / 64-127 and run
  concurrently (row tiling). exp(0.125*s) runs unmasked (scores ~N(0,1),
  no overflow), then e *= mask01 (bf16) zeroes the masked transition range.
  O^T[d,q] & Z[q] in ONE accumulating matmul per 512-query half:
  lhsT = [V_h | ones] (65 cols); raw [O^T ; Z] is DMA'd out and the host
  divides / transposes / un-sorts.

The PE is a strict FIFO, so projection matmuls for chunk p+1 are
zip-interleaved into the attention emission of pair p: the PE fills
exp-latency waits with independent projection work and the HAM clock
stays warm.

PSUM budget (8 banks): proj acc 2x[128,512] + pair scores 2x[128,1024] +
attention out 2x[65,512].

Host: transposes, sort queries by valid_len (column-suffix skipping of
fully-masked key chunks at 32-col granularity + narrow mask ranges),
bf16 0/1 mask, exact fixup of valid_len==0 rows.
"""

import os
import sys

if "/opt/trn_rl_repo" not in sys.path:
    sys.path.insert(0, "/opt/trn_rl_repo")

import numpy as np

ABLATE = set(os.environ.get("ABLATE", "").split(","))

B, S, D, H = 8, 1024, 1024, 16
DH = D // H  # 64
P = 128
HB = 512  # half-width of the query range (1 fp32 PSUM bank)
KC = S // P  # 8 key chunks
DC = D // P  # 8 hidden chunks
N_CORES = 8


def _build_nc(col_start, pred_end, reps=1):
    """col_start[kc]: first sorted-q column (mult of 32, 0..1024) needing
    key-chunk kc (1024 = chunk skipped). pred_end[kc]: end (exclusive, mult
    of 32) of the masked transition range. Unions over cores. col_start[0]
    must be 0."""
    from contextlib import ExitStack

    import concourse.mybir as mybir
    import concourse.tile as tile
    from concourse import bacc

    fp32 = mybir.dt.float32
    bf16 = mybir.dt.bfloat16
    AF = mybir.ActivationFunctionType

    nc = bacc.Bacc(
        "TRN2",
        target_bir_lowering=False,
        debug=False,
        enable_asserts=False,
        num_devices=N_CORES,
    )

    xqT = nc.dram_tensor("xqT", (D, S), bf16, kind="ExternalInput").ap()
    xkT = nc.dram_tensor("xkT", (D, S), bf16, kind="ExternalInput").ap()
    xvT = nc.dram_tensor("xvT", (D, S), bf16, kind="ExternalInput").ap()
    wq = nc.dram_tensor("wq", (D, D), bf16, kind="ExternalInput").ap()
    wk = nc.dram_tensor("wk", (D, D), bf16, kind="ExternalInput").ap()
    wv = nc.dram_tensor("wv", (D, D), bf16, kind="ExternalInput").ap()
    maskb = nc.dram_tensor("maskb", (S, S), bf16, kind="ExternalInput").ap()
    # raw per-head output: rows 0:64 = unnormalized O^T, row 64 = Z
    outT = nc.dram_tensor("outT", (H, DH + 1, S), fp32, kind="ExternalOutput").ap()

    with ExitStack() as ctx:
        tc = ctx.enter_context(tile.TileContext(nc))
        const = ctx.enter_context(tc.tile_pool(name="const", bufs=1))
        persist = ctx.enter_context(tc.tile_pool(name="persist", bufs=1))
        wpool = ctx.enter_context(tc.tile_pool(name="wpool", bufs=1))
        ppool = ctx.enter_context(tc.tile_pool(name="ppool", bufs=1, space="PSUM"))
        epool = ctx.enter_context(tc.tile_pool(name="epool", bufs=18))
        mpool = ctx.enter_context(tc.tile_pool(name="mpool", bufs=3))

        def mm(out_ap, lhsT, rhs, base, start, stop):
            # split a wide matmul into <=512-col pieces so each PE write
            # stays inside one PSUM bank. base = column offset of out_ap[0]
            # within its tile (bank alignment reference).
            w = rhs.shape[-1]
            off = 0
            while off < w:
                step = min(HB - ((base + off) % HB), w - off)
                nc.tensor.matmul(
                    out_ap[:, off : off + step],
                    lhsT,
                    rhs[:, off : off + step],
                    start=start,
                    stop=stop,
                )
                off += step

        wrm = const.tile([P, HB], bf16)
        nc.vector.memset(wrm[:], 0.5)

        rep_cm = (
            tc.For_i(0, reps, 1, hint_engines=(mybir.EngineType.PE,))
            if reps > 1
            else None
        )
        if rep_cm is not None:
            ctx.enter_context(rep_cm)

        # HAM warmup: back-to-back dummy matmuls keep the PE busy through
        # the initial input-DMA window so the clock gate opens (K=8/8)
        # before the real matmul stream starts.
        if "nowarm" not in ABLATE:
            wps = ppool.tile([P, HB], fp32, tag="sc0", name="warm")
            for _ in range(20):
                nc.tensor.matmul(wps[:], wrm[:, :P], wrm[:], start=True, stop=True)

        qt_sb = [persist.tile([P, S], bf16, tag=f"qt{i}", name=f"qt{i}") for i in range(DC)]
        kt_sb = [persist.tile([P, S], bf16, tag=f"kt{i}", name=f"kt{i}") for i in range(DC)]
        va_sb = [persist.tile([P, H * (DH + 1)], bf16, tag=f"va{i}", name=f"va{i}") for i in range(KC)]
        mk_sb = persist.tile([P, KC * S], bf16, tag="mk", name="mk")
        mk3 = mk_sb.rearrange("p (kc q) -> p kc q", q=S)

        # ---- inputs: coarse DMAs (chunk dc at cols dc*S); xv/wv split in
        # halves so the first V-projection matmuls start ~5us earlier ----
        def load_big(dram, nm, pieces=1):
            t = wpool.tile([P, DC * S], bf16, tag=nm, name=nm)
            t3 = t.rearrange("p (dc q) -> p dc q", q=S)
            d3 = dram.rearrange("(dc p) q -> p dc q", p=P)
            h = DC // pieces
            aps = []
            for i in range(pieces):
                aps.append((t3[:, i * h : (i + 1) * h, :], d3[:, i * h : (i + 1) * h, :]))
            return t, aps

        xv2, xv_aps = load_big(xvT, "xv", 2)
        wv2, wv_aps = load_big(wv, "wv", 2)
        for i in range(2):
            nc.sync.dma_start(*xv_aps[i])
            nc.sync.dma_start(*wv_aps[i])
        xk2, xk_aps = load_big(xkT, "xk")
        nc.sync.dma_start(*xk_aps[0])
        wk2, wk_aps = load_big(wk, "wk")
        nc.sync.dma_start(*wk_aps[0])
        mkb3 = maskb.rearrange("(kc p) q -> p kc q", p=P)
        nc.sync.dma_start(mk3[:, 0 : KC // 2, :], mkb3[:, 0 : KC // 2, :])
        xq2, xq_aps = load_big(xqT, "xq")
        nc.sync.dma_start(*xq_aps[0])
        wq2, wq_aps = load_big(wq, "wq")
        nc.sync.dma_start(*wq_aps[0])
        nc.sync.dma_start(mk3[:, KC // 2 : KC, :], mkb3[:, KC // 2 : KC, :])

        kcs = [kc for kc in range(KC) if col_start[kc] < S]
        nbuf = [0]
        abuf = [0]

        # ---- V projection: out[k, d] per key chunk, 512-col halves ----
        def vproj(kc):
            va3 = va_sb[kc].rearrange("p (h d) -> p h d", d=DH + 1)
            nc.vector.memset(va3[:, :, DH], 1.0)
            for half in range(2):
                acc = ppool.tile([P, HB], fp32, tag=f"pj{nbuf[0] % 2}", name="vacc")
                nbuf[0] += 1
                for dc in range(DC):
                    nc.tensor.matmul(
                        acc[:],
                        xv2[:, dc * S + kc * P : dc * S + (kc + 1) * P],
                        wv2[:, dc * D + half * HB : dc * D + (half + 1) * HB],
                        start=(dc == 0),
                        stop=(dc == DC - 1),
                    )
                nc.scalar.copy(
                    va3[:, half * (H // 2) : (half + 1) * (H // 2), 0:DH],
                    acc[:].rearrange("p (h d) -> p h d", d=DH),
                )

        def proj_ops(p):
            """Closures emitting the K+Q projections of chunk p in ~2-MM
            steps, for zip-interleaving into the attention emission."""
            ops = []
            for w2, x2, dst in ((wk2, xk2, kt_sb), (wq2, xq2, qt_sb)):
                for half in range(2):
                    box = {}
                    for dc2 in range(0, DC, 2):
                        def step(w2=w2, x2=x2, dst=dst, half=half, dc2=dc2, box=box):
                            if dc2 == 0:
                                box["acc"] = ppool.tile(
                                    [P, HB], fp32, tag=f"pj{nbuf[0] % 2}", name="acc"
                                )
                                nbuf[0] += 1
                            for dc in (dc2, dc2 + 1):
                                nc.tensor.matmul(
                                    box["acc"][:],
                                    w2[:, dc * D + p * P : dc * D + (p + 1) * P],
                                    x2[:, dc * S + half * HB : dc * S + (half + 1) * HB],
                                    start=(dc == 0),
                                    stop=(dc == DC - 1),
                                )
                            if dc2 == DC - 2:
                                nc.vector.tensor_copy(
                                    dst[p][:, half * HB : (half + 1) * HB],
                                    box["acc"][:],
                                )
                        ops.append(step)
            return ops

        def attend_pair(p, fill):
            """Heads (2p, 2p+1): row-packed scores, exp, mask-mul, AV.
            `fill` = list of closures (projection work) drained into the
            emission to keep the PE FIFO busy during exp waits."""
            oc = p
            es = {0: {}, 1: {}}
            fi = 0
            for kc in kcs:
                c0 = col_start[kc]
                cv = pred_end[kc]
                for ro in (0, 1):  # head 2p (rows 0:64), head 2p+1 (64:128)
                    sc = ppool.tile([P, S], fp32, tag=f"sc{ro}", name="sc")
                    mm(sc[:, c0:], kt_sb[oc][ro * DH : (ro + 1) * DH, kc * P : (kc + 1) * P],
                       qt_sb[oc][ro * DH : (ro + 1) * DH, c0:], c0, True, True)
                    e = epool.tile([P, S], bf16, tag="e")
                    nc.scalar.activation(e[:, c0:], sc[:, c0:], AF.Exp, scale=0.125)
                    if cv > c0 and "nopred" not in ABLATE:
                        nc.vector.tensor_mul(
                            e[:, c0:cv], e[:, c0:cv], mk3[:, kc, c0:cv]
                        )
                    es[ro][kc] = e
                for _ in range(2):
                    if fi < len(fill):
                        fill[fi]()
                        fi += 1
            while fi < len(fill):
                fill[fi]()
                fi += 1
            for ro in (0, 1):
                h = 2 * p + ro
                for half in range(2):
                    lo, hi = half * HB, (half + 1) * HB
                    ks = [kc for kc in kcs if col_start[kc] < hi]
                    at = ppool.tile(
                        [DH + 1, HB], fp32, tag=f"at{abuf[0] % 2}", name="at"
                    )
                    abuf[0] += 1
                    for j, kc in enumerate(ks):
                        c0 = max(col_start[kc], lo)
                        nc.tensor.matmul(
                            at[:, c0 - lo :],
                            va_sb[kc][:, h * (DH + 1) : (h + 1) * (DH + 1)],
                            es[ro][kc][:, c0:hi],
                            start=(j == 0),
                            stop=(j == len(ks) - 1),
                        )
                    asb = mpool.tile([DH + 1, HB], fp32, tag="asb")
                    nc.vector.tensor_copy(asb[:], at[:])
                    nc.sync.dma_start(outT[h][:, lo:hi], asb[:])

        if "noproj" not in ABLATE:
            for kc in range(KC):
                vproj(kc)
            for op in proj_ops(0):
                op()
            if "noattn" not in ABLATE:
                for p in range(1, DC):
                    attend_pair(p - 1, proj_ops(p))
                attend_pair(DC - 1, [])

    nc.compile()
    return nc


_NC_CACHE = {}
_LAST_IN_MAPS = None


def _get_nc(col_start, pred_end):
    key = (tuple(col_start), tuple(pred_end))
    if key not in _NC_CACHE:
        _NC_CACHE[key] = _build_nc(list(col_start), list(pred_end))
    return _NC_CACHE[key]


def _prep(query, key, value, valid_len, Wq, Wk, Wv):
    import ml_dtypes

    bf = ml_dtypes.bfloat16
    kidx = np.arange(S, dtype=np.int32)
    orders = []
    in_maps = []
    col_start = [S] * KC
    pred_end = [0] * KC
    wqb, wkb, wvb = Wq.astype(bf), Wk.astype(bf), Wv.astype(bf)
    for b in range(B):
        vl = valid_len[b]
        vl2 = np.where(vl == 0, 1, vl).astype(np.int32)
        order = np.argsort(vl2, kind="stable")
        orders.append(order)
        vs = vl2[order]
        for kc in range(KC):
            need = vs > (kc * P)
            c0 = S if not need.any() else (int(np.argmax(need)) // 32) * 32
            col_start[kc] = min(col_start[kc], c0)
            full = vs >= ((kc + 1) * P)
            cv = S if not full.any() else int(np.argmax(full))
            pred_end[kc] = max(pred_end[kc], min(S, -(-cv // 32) * 32))
        in_maps.append(
            {
                "xqT": np.ascontiguousarray(query[b][order].T.astype(bf)),
                "xkT": np.ascontiguousarray(key[b].T.astype(bf)),
                "xvT": np.ascontiguousarray(value[b].T.astype(bf)),
                "wq": wqb,
                "wk": wkb,
                "wv": wvb,
                "maskb": (kidx[:, None] < vs[None, :]).astype(bf),
            }
        )
    return in_maps, orders, col_start, pred_end


def kernel(query, key, value, valid_len, Wq, Wk, Wv):
    from concourse import bass_utils

    query = np.asarray(query, dtype=np.float32)
    key = np.asarray(key, dtype=np.float32)
    value = np.asarray(value, dtype=np.float32)
    valid_len = np.asarray(valid_len, dtype=np.int32)
    Wq = np.asarray(Wq, dtype=np.float32)
    Wk = np.asarray(Wk, dtype=np.float32)
    Wv = np.asarray(Wv, dtype=np.float32)

    in_maps, orders, col_start, pred_end = _prep(
        query, key, value, valid_len, Wq, Wk, Wv
    )
    nc = _get_nc(col_start, pred_end)
    global _LAST_IN_MAPS
    _LAST_IN_MAPS = in_maps
    res = bass_utils.run_bass_kernel_spmd(nc, in_maps, core_ids=list(range(N_CORES)))

    outs = np.empty((B, S, D), dtype=np.float32)
    for b in range(B):
        raw = res.results[b]["outT"]  # [H, DH+1, S] sorted-query order
        o = raw[:, :DH, :]  # [H, DH, S]
        z = raw[:, DH, :]  # [H, S]
        o_sorted = (o / z[:, None, :]).transpose(2, 0, 1).reshape(S, D)
        inv = np.empty(S, dtype=np.int64)
        inv[orders[b]] = np.arange(S)
        outs[b] = o_sorted[inv]
        zrows = np.where(valid_len[b] == 0)[0]
        if len(zrows):
            outs[b][zrows] = value[b].mean(axis=0) @ Wv
    return outs
